# revision 9
# baseline (speedup 1.0000x reference)
"""Trainium2 Bass kernel for nn_BandpassFilter — v3 (flipped-matmul design).

Math: the two order-1 IIRs summed equal (to fp32 noise) a causal FIR h[d].
Polyphase decomposition t = 128*u + p; phase-major tile xt[p, u].  The
matmul is FLIPPED vs the v1 kernel: the x window is the stationary operand
(lhsT) and the weights Wm[p, q] = h[128m + q - p] are the moving operand, so
the output lands directly in NATURAL layout [u, (c q)] — no output
transpose.  bf16 arithmetic (tolerance is 2e-2; bf16 input truncation costs
~4e-3).  Taps: lags m=0,1 full + lag 2 only for output phases q<64, worst
case 192 taps of an h that decays to ~1e-5 by then.

Cost-model facts this schedule exploits (TRN2 CoreSim):
 * DMA transfer time occupies the ISSUING engine (SP/ACT HWDGE, Pool SWDGE);
   engines are otherwise independent timelines.
 * DMA cost = per-partition-bytes * 0.386ns, 2x if the min contiguous run
   < 512B, floor 500ns.  gpsimd casting DMAs are billed on OUTPUT bytes:
   f32->bf16 cast-in costs the same as f32-in (the 2x small-run penalty
   cancels the byte halving) but lands bf16 for 1cyc/col PE transposes.
 * PE: bf16 matmul/transpose 1 cyc/output-col; f32r transpose 1.5; fp32 2.
 * XBAR DmaTransposeAnt (bf16, SP/ACT): ~163ns per 128x128 chunk.
 * bf16 PSUM->SBUF copies can be bitcast to f32 for half the DVE cycles.

Per-row strategies, mixed to balance the 5 engine timelines:
  F: f32-in halves on SP+ACT, PE f32r transposes, DVE copy casts to bf16
  C: bf16 cast-in on Pool,    PE bf16 transposes, DVE copy (f32-bitcast)
  X: bf16 cast-in on Pool,    XBAR transposes on SP/ACT (no PSUM copy)

Sharding: batch dim (64 waveforms) split 8 ways across 8 NeuronCores.
"""

import numpy as np

SAMPLE_RATE = 44100.0
B_FULL = 64
T = 262144
NCORES = 8
RPC = B_FULL // NCORES  # rows (waveforms) per core
P = 128                 # phases == partitions
J = T // P              # 2048 phase-major columns per row
NCH = J // P            # 16 column-chunks of 128
PAD = 4                 # left zero-padding columns (>= 2, even for bitcast)
NLAGS = 3
LOWQ = 0                # lag-2 covers output phases q < LOWQ (0: lag-2 off)
WARMUP = 0              # dummy PE matmuls (no effect; p-state resets on idle)
ROW0_Q = 4              # row-0 input DMA granules
FINE_TAIL = False       # eighth-granules for the final bank
TAIL_CP = "VA"          # tail eighth copy engines
TAIL_OUT = "SG"         # tail eighth out engines
CONV_G = (1, 3)         # which transpose-slot each conv half is emitted at
UNPAIR0 = False         # row 0 unpaired psi start (hurts; keep paired)
DEEP_TG = ()            # rows whose transposes are emitted 2 rounds early
F_IN = {1: "S", 5: "A"}  # F-row input DMA engine
PSI_ENG = "VAVVVAVV"    # psi PSUM->SBUF copy engine per row (V/A)

# per-row input/transpose strategy
ROW_STRATEGY = "CFCCCFCC"
# psy (conv PSUM->SBUF) copy engine per (row, half): V=DVE, A=ACT, G=Pool
PSY_ENG = ["VA", "AV", "VA", "AV", "VA", "AV", "VA", "AV"]
# out-DMA quarter engines per row: S=SP, A=ACT, G=Pool
OUT_ENG = ["SSSA", "SSSA", "SSSA", "SASA", "SSSA", "SAGA", "SGSG", "SGAS"]


def _coeffs(low_cutoff, high_cutoff):
    f32 = np.float32
    nyq = f32(SAMPLE_RATE / 2.0)
    low = np.clip(f32(low_cutoff), f32(0.0), nyq)
    high = np.clip(f32(high_cutoff), low, nyq)

    def butter1(wn, btype):
        t = np.tan(f32(np.pi) * wn / f32(2.0))
        a1 = (t - f32(1.0)) / (t + f32(1.0))
        if btype == "low":
            b0 = t / (f32(1.0) + t)
            b1 = b0
        else:
            b0 = f32(1.0) / (f32(1.0) + t)
            b1 = -b0
        return b0, b1, a1

    bh0, bh1, ah1 = butter1(low / nyq, "high")
    bl0, bl1, al1 = butter1(high / nyq, "low")
    return (bh0, bh1, ah1), (bl0, bl1, al1)


def _impulse_response(low_cutoff, high_cutoff, n):
    (bh0, bh1, ah1), (bl0, bl1, al1) = _coeffs(low_cutoff, high_cutoff)
    Ah, Al = -np.float64(ah1), -np.float64(al1)
    ch = np.float64(bh1) - np.float64(ah1) * np.float64(bh0)
    cl = np.float64(bl1) - np.float64(al1) * np.float64(bl0)
    d = np.arange(1, n)
    h = np.empty(n, np.float64)
    h[0] = np.float64(bh0) + np.float64(bl0)
    h[1:] = ch * Ah ** (d - 1) + cl * Al ** (d - 1)
    return h


def _weights(low_cutoff, high_cutoff):
    """Flipped-layout weights: w[m, p, q] = h[128*m + q - p]."""
    h = _impulse_response(low_cutoff, high_cutoff, NLAGS * P)
    q = np.arange(P)[None, :]
    p = np.arange(P)[:, None]
    w = np.zeros((NLAGS, P, P), np.float64)
    for m in range(NLAGS):
        d = P * m + q - p
        valid = d >= 0
        w[m][valid] = h[d[valid]]
    return w.astype(np.float32)


_BUILD_CACHE = {}


def _legalize_waits(nc, mybir):
    """The walrus build accepts at most ONE sync-wait per instruction; split
    extras into standalone EventSemaphore instructions on the same queue."""
    n = 0
    for fn in nc.m.functions:
        for blk in fn.blocks:
            new = []
            for inst in blk.instructions:
                si = getattr(inst, "sync_info", None)
                if si is not None and si.on_wait and len(si.on_wait) > 1:
                    waits = list(si.on_wait)
                    for w in waits[:-1]:
                        n += 1
                        new.append(mybir.InstEventSemaphore(
                            name=f"wsplit-{n}-{inst.name}",
                            engine=inst.engine,
                            ins=[], outs=[],
                            sync_info=mybir.SyncInfo(on_wait=[w],
                                                     on_update=[]),
                        ))
                    inst.sync_info = mybir.SyncInfo(
                        on_wait=[waits[-1]],
                        on_update=list(si.on_update or []))
                new.append(inst)
            blk.instructions = new
    return n


def build_nc(reps=1, legalize=True, loop_n=1):
    key = (reps, legalize, loop_n)
    if key in _BUILD_CACHE:
        return _BUILD_CACHE[key]

    import concourse.bass as bass
    import concourse.mybir as mybir
    from concourse import tile
    from contextlib import ExitStack

    f32 = mybir.dt.float32
    f32r = mybir.dt.float32r
    bf16 = mybir.dt.bfloat16

    nc = bass.Bass()
    x_in = nc.declare_dram_parameter("x", [RPC, T], f32, isOutput=False)
    w_in = nc.declare_dram_parameter("w", [NLAGS, P, P], f32, isOutput=False)
    id_in = nc.declare_dram_parameter("ident", [P, P], f32, isOutput=False)
    y_out = nc.declare_dram_parameter("y", [RPC, T], f32, isOutput=True)

    with tile.TileContext(nc) as tc, ExitStack() as ctx:
        const = ctx.enter_context(tc.tile_pool(name="const", bufs=1))
        xn_pool = ctx.enter_context(tc.tile_pool(name="xn", bufs=4))
        xf_pool = ctx.enter_context(tc.tile_pool(name="xf", bufs=2))
        xt_pool = ctx.enter_context(tc.tile_pool(name="xt", bufs=4))
        yn_pool = ctx.enter_context(tc.tile_pool(name="yn", bufs=3))
        psi_pool = ctx.enter_context(
            tc.tile_pool(name="psi", bufs=2, space="PSUM"))
        psy_pool = ctx.enter_context(
            tc.tile_pool(name="psy", bufs=3, space="PSUM"))

        wf = const.tile([P, NLAGS * P], f32)
        wb = const.tile([P, NLAGS * P], bf16)
        idf = const.tile([P, P], f32)
        idb = const.tile([P, P], bf16)

        idr = const.tile([P, P], f32r)

        def load_consts():
            nc.scalar.dma_start(
                out=wf[:].rearrange("p (m q) -> p m q", q=P),
                in_=w_in.rearrange("m p q -> p m q"))
            nc.scalar.dma_start(out=idf[:], in_=id_in[:])
            nc.vector.tensor_copy(wb[:], wf[:])
            nc.vector.tensor_copy(idb[:], idf[:])
            nc.vector.tensor_copy(idr[:], idf[:])

        if WARMUP:
            warm = const.tile([P, P], bf16)
            wps = psi_pool.tile([P, 256], f32, tag="psi")
            nc.vector.memset(warm[:], 0.0)
            for _ in range(WARMUP):
                nc.tensor.matmul(wps[:, 0:P], warm[:], warm[:],
                                 start=True, stop=True)

        hc = NCH // 2
        eng_of = {"S": nc.sync, "A": nc.scalar, "G": nc.gpsimd}

        def copy_psy(eng, dst, src):
            if eng == "V":
                nc.vector.tensor_copy(dst, src)
            elif eng == "A":
                nc.scalar.copy(dst, src)
            else:
                nc.gpsimd.tensor_copy(dst, src)

        if loop_n > 1:
            ctx.enter_context(tc.For_i(0, loop_n, 1, staggered_reset=True))
        for rep in range(reps):
            xnb_t, xt_t, yn_t = {}, {}, {}

            def stage_in(r):
                strat = ROW_STRATEGY[r]
                xr3 = x_in[r].rearrange("(c u p) -> u c p", u=P, p=P)
                splits = ([2, 2, 4, 4, 4] if r == 0 else [8, 8])
                if strat == "F":
                    xnb = xf_pool.tile([P, J], f32r, tag="xf")
                    xn3 = xnb[:].rearrange("u (c p) -> u c p", p=P)
                    xr3r = xr3.bitcast(f32r)
                    feng = eng_of[F_IN.get(r, "S")]
                    c0 = 0
                    for w in splits:
                        feng.dma_start(out=xn3[:, c0:c0 + w],
                                       in_=xr3r[:, c0:c0 + w])
                        c0 += w
                else:
                    xnb = xn_pool.tile([P, J], bf16, tag="xn")
                    xn3 = xnb[:].rearrange("u (c p) -> u c p", p=P)
                    c0 = 0
                    for w in splits:
                        nc.gpsimd.dma_start(out=xn3[:, c0:c0 + w],
                                            in_=xr3[:, c0:c0 + w])
                        c0 += w
                xnb_t[r] = xnb
                xt = xt_pool.tile([P, PAD + J], bf16, tag="xt")
                xt_t[r] = xt
                nc.vector.memset(xt[:, 0:PAD], 0.0)

            def transpose_group(r, g):
                strat = ROW_STRATEGY[r]
                xnb, xt = xnb_t[r], xt_t[r]
                if strat == "X":
                    for k in range(4):
                        c = g * 4 + k
                        eng = nc.sync if (c % 2 == 0) else nc.scalar
                        eng.dma_start(
                            out=xt[:, PAD + c * P:PAD + (c + 1) * P],
                            in_=xnb[:, c * P:(c + 1) * P],
                            transpose=True)
                elif strat == "C":
                    if r == 0 and UNPAIR0:
                        psi = psi_pool.tile([P, 512], bf16, tag="psi")
                        for k in range(4):
                            c = g * 4 + k
                            nc.tensor.transpose(
                                psi[:, k * P:(k + 1) * P],
                                xnb[:, c * P:(c + 1) * P], idb[:])
                        dst = xt[:, PAD + g * 512:
                                 PAD + (g + 1) * 512].bitcast(f32)
                        nc.vector.tensor_copy(dst, psi[:].bitcast(f32))
                        return
                    if g % 2 == 1:
                        return
                    psi = psi_pool.tile([P, 1024], bf16, tag="psi")
                    for k in range(8):
                        c = g * 4 + k
                        nc.tensor.transpose(
                            psi[:, k * P:(k + 1) * P],
                            xnb[:, c * P:(c + 1) * P], idb[:])
                    # bf16 data moved as bitcast f32: half the DVE cycles
                    dst = xt[:, PAD + g * 512:PAD + (g + 2) * 512].bitcast(f32)
                    if PSI_ENG[r] == "V":
                        nc.vector.tensor_copy(dst, psi[:].bitcast(f32))
                    else:
                        nc.scalar.copy(dst, psi[:].bitcast(f32))
                else:  # F: f32r transpose, copy casts f32r->bf16
                    psi = psi_pool.tile([P, 512], f32r, tag="psi")
                    for k in range(4):
                        c = g * 4 + k
                        nc.tensor.transpose(
                            psi[:, k * P:(k + 1) * P],
                            xnb[:, c * P:(c + 1) * P],
                            idr[:])
                    dstf = xt[:, PAD + g * 512:PAD + (g + 1) * 512]
                    if PSI_ENG[r] == "V":
                        nc.vector.tensor_copy(dstf, psi[:].bitcast(f32))
                    else:
                        nc.scalar.copy(dstf, psi[:].bitcast(f32))

            def conv_half(r, h):
                """Half-row conv: 8 chunks into a 2-bank [128,1024] psy tile,
                one PSUM->SBUF copy."""
                xt = xt_t[r]
                if h == 0:
                    yn = yn_pool.tile([P, J], f32, tag="yn")
                    yn_t[r] = yn
                yn = yn_t[r]
                psy = psy_pool.tile([P, 1024], f32, tag="psy")
                last = r == RPC - 1
                for k in range(8):
                    c = h * 8 + k
                    b0 = PAD + c * P
                    nc.tensor.matmul(
                        psy[:, k * P:(k + 1) * P],
                        xt[:, b0:b0 + P], wb[:, 0:P],
                        start=True, stop=False)
                    if LOWQ:
                        nc.tensor.matmul(
                            psy[:, k * P:k * P + LOWQ],
                            xt[:, b0 - 2:b0 - 2 + P],
                            wb[:, 2 * P:2 * P + LOWQ],
                            start=False, stop=False)
                    nc.tensor.matmul(
                        psy[:, k * P:(k + 1) * P],
                        xt[:, b0 - 1:b0 - 1 + P], wb[:, P:2 * P],
                        start=False, stop=True)
                    if last and k in (3, 7):
                        b = k // 4
                        q = 2 * h + b
                        fine = FINE_TAIL
                        yo3 = y_out[r].rearrange("(c u p) -> u c p",
                                                 u=P, p=P)
                        yn3 = yn[:].rearrange("u (c p) -> u c p", p=P)
                        if q < 3 or not fine:
                            col = q * 512
                            qq4 = q
                            copy_psy("VAVA"[q], yn[:, col:col + 512],
                                     psy[:, b * 512:(b + 1) * 512])
                            eng_of["SAGS"[q]].dma_start(
                                out=yo3[:, qq4 * 4:(qq4 + 1) * 4],
                                in_=yn3[:, qq4 * 4:(qq4 + 1) * 4])
                        else:
                            for e in range(2):
                                col = 1536 + e * 256
                                copy_psy(TAIL_CP[e], yn[:, col:col + 256],
                                         psy[:, 512 + e * 256:768 + e * 256])
                                eng_of[TAIL_OUT[e]].dma_start(
                                    out=yo3[:, 12 + e * 2:14 + e * 2],
                                    in_=yn3[:, 12 + e * 2:14 + e * 2])
                if not last:
                    copy_psy(PSY_ENG[r][h],
                             yn[:, h * 1024:(h + 1) * 1024], psy[:])

            def stage_out(r):
                yn = yn_t[r]
                qc = NCH // 4
                yo3 = y_out[r].rearrange("(c u p) -> u c p", u=P, p=P)
                yn3 = yn[:].rearrange("u (c p) -> u c p", p=P)
                for q in range(4):
                    eng_of[OUT_ENG[r][q]].dma_start(
                        out=yo3[:, q * qc:(q + 1) * qc],
                        in_=yn3[:, q * qc:(q + 1) * qc])

            # software-pipelined emission, 3-row input skew.  X rows get
            # their XBAR transposes TWO rounds early so the 900ns DMA-sem
            # propagation hides behind a full row of compute.
            def tg_round(rr):
                if ROW_STRATEGY[rr] == "X" or rr in DEEP_TG:
                    return rr - 2
                return rr - 1

            stage_in(0)
            stage_in(1)
            stage_in(2)
            load_consts()
            for g in range(4):
                for rr in range(RPC):
                    if tg_round(rr) < 0:
                        transpose_group(rr, g)
            for r in range(RPC):
                if r + 3 < RPC:
                    stage_in(r + 3)
                for g in range(4):
                    for rr in range(r + 1, min(r + 3, RPC)):
                        if tg_round(rr) == r:
                            transpose_group(rr, g)
                    if g == CONV_G[0]:
                        conv_half(r, 0)
                    elif g == CONV_G[1]:
                        conv_half(r, 1)
                if r < RPC - 1:
                    stage_out(r)

    if legalize:
        _legalize_waits(nc, mybir)
    _BUILD_CACHE[key] = nc
    return nc


def kernel(x, low_cutoff, high_cutoff):
    from concourse.bass_utils import run_bass_kernel_spmd

    x = np.asarray(x, dtype=np.float32)
    w = _weights(np.asarray(low_cutoff), np.asarray(high_cutoff))
    ident = np.eye(P, dtype=np.float32)

    nc = build_nc(reps=1)
    in_maps = [
        {"x": np.ascontiguousarray(x[c * RPC:(c + 1) * RPC]),
         "w": w, "ident": ident}
        for c in range(NCORES)
    ]
    res = run_bass_kernel_spmd(nc, in_maps, list(range(NCORES)))
    return np.concatenate([res.results[c]["y"] for c in range(NCORES)], axis=0)


# revision 10
# speedup vs baseline: 1.0023x; 1.0023x over previous
"""Trainium2 Bass kernel for nn_BandpassFilter — v3 (flipped-matmul design).

Math: the two order-1 IIRs summed equal (to fp32 noise) a causal FIR h[d].
Polyphase decomposition t = 128*u + p; phase-major tile xt[p, u].  The
matmul is FLIPPED vs the v1 kernel: the x window is the stationary operand
(lhsT) and the weights Wm[p, q] = h[128m + q - p] are the moving operand, so
the output lands directly in NATURAL layout [u, (c q)] — no output
transpose.  bf16 arithmetic (tolerance is 2e-2; bf16 input truncation costs
~4e-3).  Taps: lags m=0,1 full + lag 2 only for output phases q<64, worst
case 192 taps of an h that decays to ~1e-5 by then.

Cost-model facts this schedule exploits (TRN2 CoreSim):
 * DMA transfer time occupies the ISSUING engine (SP/ACT HWDGE, Pool SWDGE);
   engines are otherwise independent timelines.
 * DMA cost = per-partition-bytes * 0.386ns, 2x if the min contiguous run
   < 512B, floor 500ns.  gpsimd casting DMAs are billed on OUTPUT bytes:
   f32->bf16 cast-in costs the same as f32-in (the 2x small-run penalty
   cancels the byte halving) but lands bf16 for 1cyc/col PE transposes.
 * PE: bf16 matmul/transpose 1 cyc/output-col; f32r transpose 1.5; fp32 2.
 * XBAR DmaTransposeAnt (bf16, SP/ACT): ~163ns per 128x128 chunk.
 * bf16 PSUM->SBUF copies can be bitcast to f32 for half the DVE cycles.

Per-row strategies, mixed to balance the 5 engine timelines:
  F: f32-in halves on SP+ACT, PE f32r transposes, DVE copy casts to bf16
  C: bf16 cast-in on Pool,    PE bf16 transposes, DVE copy (f32-bitcast)
  X: bf16 cast-in on Pool,    XBAR transposes on SP/ACT (no PSUM copy)

Sharding: batch dim (64 waveforms) split 8 ways across 8 NeuronCores.
"""

import numpy as np

SAMPLE_RATE = 44100.0
B_FULL = 64
T = 262144
NCORES = 8
RPC = B_FULL // NCORES  # rows (waveforms) per core
P = 128                 # phases == partitions
J = T // P              # 2048 phase-major columns per row
NCH = J // P            # 16 column-chunks of 128
PAD = 4                 # left zero-padding columns (>= 2, even for bitcast)
NLAGS = 3
LOWQ = 0                # lag-2 covers output phases q < LOWQ (0: lag-2 off)
WARMUP = 0              # dummy PE matmuls (no effect; p-state resets on idle)
ROW0_Q = 4              # row-0 input DMA granules
FINE_TAIL = False       # eighth-granules for the final bank
TAIL_CP = "VA"          # tail eighth copy engines
TAIL_OUT = "SG"         # tail eighth out engines
CONV_G = (1, 3)         # which transpose-slot each conv half is emitted at
PSY_FINE = False        # conv in 4 quarter-units/row with 6-buf [P,512] psy
UNPAIR0 = False         # row 0 unpaired psi start (hurts; keep paired)
DEEP_TG = ()            # rows whose transposes are emitted 2 rounds early
F_IN = {1: "S", 5: "A"}  # F-row input DMA engine
PSI_ENG = "VAVVVAVV"    # psi PSUM->SBUF copy engine per row (V/A)

# per-row input/transpose strategy
ROW_STRATEGY = "CFCCCFCC"
# psy (conv PSUM->SBUF) copy engine per (row, half): V=DVE, A=ACT, G=Pool
PSY_ENG = ["VA", "AV", "VA", "AV", "VA", "AV", "VA", "AV"]
# out-DMA quarter engines per row: S=SP, A=ACT, G=Pool
OUT_ENG = ["SSSA", "SSSA", "SSSA", "SASA", "SSSA", "SAGA", "SGSG", "SGAS"]


def _coeffs(low_cutoff, high_cutoff):
    f32 = np.float32
    nyq = f32(SAMPLE_RATE / 2.0)
    low = np.clip(f32(low_cutoff), f32(0.0), nyq)
    high = np.clip(f32(high_cutoff), low, nyq)

    def butter1(wn, btype):
        t = np.tan(f32(np.pi) * wn / f32(2.0))
        a1 = (t - f32(1.0)) / (t + f32(1.0))
        if btype == "low":
            b0 = t / (f32(1.0) + t)
            b1 = b0
        else:
            b0 = f32(1.0) / (f32(1.0) + t)
            b1 = -b0
        return b0, b1, a1

    bh0, bh1, ah1 = butter1(low / nyq, "high")
    bl0, bl1, al1 = butter1(high / nyq, "low")
    return (bh0, bh1, ah1), (bl0, bl1, al1)


def _impulse_response(low_cutoff, high_cutoff, n):
    (bh0, bh1, ah1), (bl0, bl1, al1) = _coeffs(low_cutoff, high_cutoff)
    Ah, Al = -np.float64(ah1), -np.float64(al1)
    ch = np.float64(bh1) - np.float64(ah1) * np.float64(bh0)
    cl = np.float64(bl1) - np.float64(al1) * np.float64(bl0)
    d = np.arange(1, n)
    h = np.empty(n, np.float64)
    h[0] = np.float64(bh0) + np.float64(bl0)
    h[1:] = ch * Ah ** (d - 1) + cl * Al ** (d - 1)
    return h


def _weights(low_cutoff, high_cutoff):
    """Flipped-layout weights: w[m, p, q] = h[128*m + q - p]."""
    h = _impulse_response(low_cutoff, high_cutoff, NLAGS * P)
    q = np.arange(P)[None, :]
    p = np.arange(P)[:, None]
    w = np.zeros((NLAGS, P, P), np.float64)
    for m in range(NLAGS):
        d = P * m + q - p
        valid = d >= 0
        w[m][valid] = h[d[valid]]
    return w.astype(np.float32)


_BUILD_CACHE = {}


def _legalize_waits(nc, mybir):
    """The walrus build accepts at most ONE sync-wait per instruction; split
    extras into standalone EventSemaphore instructions on the same queue."""
    n = 0
    for fn in nc.m.functions:
        for blk in fn.blocks:
            new = []
            for inst in blk.instructions:
                si = getattr(inst, "sync_info", None)
                if si is not None and si.on_wait and len(si.on_wait) > 1:
                    waits = list(si.on_wait)
                    for w in waits[:-1]:
                        n += 1
                        new.append(mybir.InstEventSemaphore(
                            name=f"wsplit-{n}-{inst.name}",
                            engine=inst.engine,
                            ins=[], outs=[],
                            sync_info=mybir.SyncInfo(on_wait=[w],
                                                     on_update=[]),
                        ))
                    inst.sync_info = mybir.SyncInfo(
                        on_wait=[waits[-1]],
                        on_update=list(si.on_update or []))
                new.append(inst)
            blk.instructions = new
    return n


def build_nc(reps=1, legalize=True, loop_n=1):
    key = (reps, legalize, loop_n)
    if key in _BUILD_CACHE:
        return _BUILD_CACHE[key]

    import concourse.bass as bass
    import concourse.mybir as mybir
    from concourse import tile
    from contextlib import ExitStack

    f32 = mybir.dt.float32
    f32r = mybir.dt.float32r
    bf16 = mybir.dt.bfloat16

    nc = bass.Bass()
    x_in = nc.declare_dram_parameter("x", [RPC, T], f32, isOutput=False)
    w_in = nc.declare_dram_parameter("w", [NLAGS, P, P], f32, isOutput=False)
    id_in = nc.declare_dram_parameter("ident", [P, P], f32, isOutput=False)
    y_out = nc.declare_dram_parameter("y", [RPC, T], f32, isOutput=True)

    with tile.TileContext(nc) as tc, ExitStack() as ctx:
        const = ctx.enter_context(tc.tile_pool(name="const", bufs=1))
        xn_pool = ctx.enter_context(tc.tile_pool(name="xn", bufs=4))
        xf_pool = ctx.enter_context(tc.tile_pool(name="xf", bufs=2))
        xt_pool = ctx.enter_context(tc.tile_pool(name="xt", bufs=4))
        yn_pool = ctx.enter_context(tc.tile_pool(name="yn", bufs=3))
        psi_pool = ctx.enter_context(
            tc.tile_pool(name="psi", bufs=2, space="PSUM"))
        psy_pool = ctx.enter_context(
            tc.tile_pool(name="psy", bufs=6 if PSY_FINE else 3,
                         space="PSUM"))

        wf = const.tile([P, NLAGS * P], f32)
        wb = const.tile([P, NLAGS * P], bf16)
        idf = const.tile([P, P], f32)
        idb = const.tile([P, P], bf16)

        idr = const.tile([P, P], f32r)

        def load_consts():
            nc.scalar.dma_start(out=idf[:], in_=id_in[:])
            nc.scalar.dma_start(
                out=wf[:].rearrange("p (m q) -> p m q", q=P),
                in_=w_in.rearrange("m p q -> p m q"))
            nc.vector.tensor_copy(idb[:], idf[:])
            nc.vector.tensor_copy(idr[:], idf[:])
            nc.vector.tensor_copy(wb[:], wf[:])

        if WARMUP:
            warm = const.tile([P, P], bf16)
            wps = psi_pool.tile([P, 256], f32, tag="psi")
            nc.vector.memset(warm[:], 0.0)
            for _ in range(WARMUP):
                nc.tensor.matmul(wps[:, 0:P], warm[:], warm[:],
                                 start=True, stop=True)

        hc = NCH // 2
        eng_of = {"S": nc.sync, "A": nc.scalar, "G": nc.gpsimd}

        def copy_psy(eng, dst, src):
            if eng == "V":
                nc.vector.tensor_copy(dst, src)
            elif eng == "A":
                nc.scalar.copy(dst, src)
            else:
                nc.gpsimd.tensor_copy(dst, src)

        if loop_n > 1:
            ctx.enter_context(tc.For_i(0, loop_n, 1, staggered_reset=True))
        for rep in range(reps):
            xnb_t, xt_t, yn_t = {}, {}, {}

            def stage_in(r):
                strat = ROW_STRATEGY[r]
                xr3 = x_in[r].rearrange("(c u p) -> u c p", u=P, p=P)
                splits = ([2, 2, 4, 4, 4] if r == 0 else [8, 8])
                if strat == "F":
                    xnb = xf_pool.tile([P, J], f32r, tag="xf")
                    xn3 = xnb[:].rearrange("u (c p) -> u c p", p=P)
                    xr3r = xr3.bitcast(f32r)
                    feng = eng_of[F_IN.get(r, "S")]
                    c0 = 0
                    for w in splits:
                        feng.dma_start(out=xn3[:, c0:c0 + w],
                                       in_=xr3r[:, c0:c0 + w])
                        c0 += w
                else:
                    xnb = xn_pool.tile([P, J], bf16, tag="xn")
                    xn3 = xnb[:].rearrange("u (c p) -> u c p", p=P)
                    c0 = 0
                    for w in splits:
                        nc.gpsimd.dma_start(out=xn3[:, c0:c0 + w],
                                            in_=xr3[:, c0:c0 + w])
                        c0 += w
                xnb_t[r] = xnb
                xt = xt_pool.tile([P, PAD + J], bf16, tag="xt")
                xt_t[r] = xt
                nc.vector.memset(xt[:, 0:PAD], 0.0)

            def transpose_group(r, g):
                strat = ROW_STRATEGY[r]
                xnb, xt = xnb_t[r], xt_t[r]
                if strat == "X":
                    for k in range(4):
                        c = g * 4 + k
                        eng = nc.sync if (c % 2 == 0) else nc.scalar
                        eng.dma_start(
                            out=xt[:, PAD + c * P:PAD + (c + 1) * P],
                            in_=xnb[:, c * P:(c + 1) * P],
                            transpose=True)
                elif strat == "C":
                    if r == 0 and UNPAIR0:
                        psi = psi_pool.tile([P, 512], bf16, tag="psi")
                        for k in range(4):
                            c = g * 4 + k
                            nc.tensor.transpose(
                                psi[:, k * P:(k + 1) * P],
                                xnb[:, c * P:(c + 1) * P], idb[:])
                        dst = xt[:, PAD + g * 512:
                                 PAD + (g + 1) * 512].bitcast(f32)
                        nc.vector.tensor_copy(dst, psi[:].bitcast(f32))
                        return
                    if g % 2 == 1:
                        return
                    psi = psi_pool.tile([P, 1024], bf16, tag="psi")
                    for k in range(8):
                        c = g * 4 + k
                        nc.tensor.transpose(
                            psi[:, k * P:(k + 1) * P],
                            xnb[:, c * P:(c + 1) * P], idb[:])
                    # bf16 data moved as bitcast f32: half the DVE cycles
                    dst = xt[:, PAD + g * 512:PAD + (g + 2) * 512].bitcast(f32)
                    if PSI_ENG[r] == "V":
                        nc.vector.tensor_copy(dst, psi[:].bitcast(f32))
                    else:
                        nc.scalar.copy(dst, psi[:].bitcast(f32))
                else:  # F: f32r transpose, copy casts f32r->bf16
                    psi = psi_pool.tile([P, 512], f32r, tag="psi")
                    for k in range(4):
                        c = g * 4 + k
                        nc.tensor.transpose(
                            psi[:, k * P:(k + 1) * P],
                            xnb[:, c * P:(c + 1) * P],
                            idr[:])
                    dstf = xt[:, PAD + g * 512:PAD + (g + 1) * 512]
                    if PSI_ENG[r] == "V":
                        nc.vector.tensor_copy(dstf, psi[:].bitcast(f32))
                    else:
                        nc.scalar.copy(dstf, psi[:].bitcast(f32))

            def conv_half(r, h):
                """Half-row conv: 8 chunks into a 2-bank [128,1024] psy tile,
                one PSUM->SBUF copy."""
                xt = xt_t[r]
                if h == 0:
                    yn = yn_pool.tile([P, J], f32, tag="yn")
                    yn_t[r] = yn
                yn = yn_t[r]
                psy = psy_pool.tile([P, 1024], f32, tag="psy")
                last = r == RPC - 1
                for k in range(8):
                    c = h * 8 + k
                    b0 = PAD + c * P
                    nc.tensor.matmul(
                        psy[:, k * P:(k + 1) * P],
                        xt[:, b0:b0 + P], wb[:, 0:P],
                        start=True, stop=False)
                    if LOWQ:
                        nc.tensor.matmul(
                            psy[:, k * P:k * P + LOWQ],
                            xt[:, b0 - 2:b0 - 2 + P],
                            wb[:, 2 * P:2 * P + LOWQ],
                            start=False, stop=False)
                    nc.tensor.matmul(
                        psy[:, k * P:(k + 1) * P],
                        xt[:, b0 - 1:b0 - 1 + P], wb[:, P:2 * P],
                        start=False, stop=True)
                    if last and k in (3, 7):
                        b = k // 4
                        q = 2 * h + b
                        fine = FINE_TAIL
                        yo3 = y_out[r].rearrange("(c u p) -> u c p",
                                                 u=P, p=P)
                        yn3 = yn[:].rearrange("u (c p) -> u c p", p=P)
                        if q < 3 or not fine:
                            col = q * 512
                            qq4 = q
                            copy_psy("VAVA"[q], yn[:, col:col + 512],
                                     psy[:, b * 512:(b + 1) * 512])
                            eng_of["SAGS"[q]].dma_start(
                                out=yo3[:, qq4 * 4:(qq4 + 1) * 4],
                                in_=yn3[:, qq4 * 4:(qq4 + 1) * 4])
                        else:
                            for e in range(2):
                                col = 1536 + e * 256
                                copy_psy(TAIL_CP[e], yn[:, col:col + 256],
                                         psy[:, 512 + e * 256:768 + e * 256])
                                eng_of[TAIL_OUT[e]].dma_start(
                                    out=yo3[:, 12 + e * 2:14 + e * 2],
                                    in_=yn3[:, 12 + e * 2:14 + e * 2])
                if not last:
                    copy_psy(PSY_ENG[r][h],
                             yn[:, h * 1024:(h + 1) * 1024], psy[:])

            def conv_quarter(r, qq):
                """Quarter-row conv: 4 chunks into a [P,512] psy tile."""
                xt = xt_t[r]
                if qq == 0:
                    yn = yn_pool.tile([P, J], f32, tag="yn")
                    yn_t[r] = yn
                yn = yn_t[r]
                psy = psy_pool.tile([P, 512], f32, tag="psy")
                last = r == RPC - 1
                for k in range(4):
                    c = qq * 4 + k
                    b0 = PAD + c * P
                    nc.tensor.matmul(
                        psy[:, k * P:(k + 1) * P],
                        xt[:, b0:b0 + P], wb[:, 0:P],
                        start=True, stop=False)
                    nc.tensor.matmul(
                        psy[:, k * P:(k + 1) * P],
                        xt[:, b0 - 1:b0 - 1 + P], wb[:, P:2 * P],
                        start=False, stop=True)
                col = qq * 512
                eng = PSY_ENG[r][qq // 2] if not last else "VAVA"[qq]
                copy_psy(eng, yn[:, col:col + 512], psy[:])
                if last:
                    yo3 = y_out[r].rearrange("(c u p) -> u c p", u=P, p=P)
                    yn3 = yn[:].rearrange("u (c p) -> u c p", p=P)
                    eng_of["SAGS"[qq]].dma_start(
                        out=yo3[:, qq * 4:(qq + 1) * 4],
                        in_=yn3[:, qq * 4:(qq + 1) * 4])

            def stage_out(r):
                yn = yn_t[r]
                qc = NCH // 4
                yo3 = y_out[r].rearrange("(c u p) -> u c p", u=P, p=P)
                yn3 = yn[:].rearrange("u (c p) -> u c p", p=P)
                for q in range(4):
                    eng_of[OUT_ENG[r][q]].dma_start(
                        out=yo3[:, q * qc:(q + 1) * qc],
                        in_=yn3[:, q * qc:(q + 1) * qc])

            # software-pipelined emission, 3-row input skew.  X rows get
            # their XBAR transposes TWO rounds early so the 900ns DMA-sem
            # propagation hides behind a full row of compute.
            def tg_round(rr):
                if ROW_STRATEGY[rr] == "X" or rr in DEEP_TG:
                    return rr - 2
                return rr - 1

            stage_in(0)
            stage_in(1)
            stage_in(2)
            load_consts()
            for g in range(4):
                for rr in range(RPC):
                    if tg_round(rr) < 0:
                        transpose_group(rr, g)
            for r in range(RPC):
                if r + 3 < RPC:
                    stage_in(r + 3)
                for g in range(4):
                    for rr in range(r + 1, min(r + 3, RPC)):
                        if tg_round(rr) == r:
                            transpose_group(rr, g)
                    if PSY_FINE:
                        conv_quarter(r, g)
                    elif g == CONV_G[0]:
                        conv_half(r, 0)
                    elif g == CONV_G[1]:
                        conv_half(r, 1)
                if r < RPC - 1:
                    stage_out(r)

    if legalize:
        _legalize_waits(nc, mybir)
    _BUILD_CACHE[key] = nc
    return nc


def kernel(x, low_cutoff, high_cutoff):
    from concourse.bass_utils import run_bass_kernel_spmd

    x = np.asarray(x, dtype=np.float32)
    w = _weights(np.asarray(low_cutoff), np.asarray(high_cutoff))
    ident = np.eye(P, dtype=np.float32)

    nc = build_nc(reps=1)
    in_maps = [
        {"x": np.ascontiguousarray(x[c * RPC:(c + 1) * RPC]),
         "w": w, "ident": ident}
        for c in range(NCORES)
    ]
    res = run_bass_kernel_spmd(nc, in_maps, list(range(NCORES)))
    return np.concatenate([res.results[c]["y"] for c in range(NCORES)], axis=0)
